# revision 26
# baseline (speedup 1.0000x reference)
"""Trainium2 Bass kernel for nn_Decoder: 2-layer GRU decoder with argmax feedback.

v2 design (latency-oriented):
- Pure data parallel: 8 cores x 1024 batch. Within a core, the batch is split
  into G=2 independent groups of 512, each with its own argmax->input feedback
  chain; the tile scheduler pipelines the two chains across engines.
- State transposed on-chip: h0,h1 as [H=128 partitions, B free] (f32r).
- Layer-0 input projection via 32-row gather matmul GI.T @ onehotT (GI
  precomputed on host with folded biases).
- h-update split trick: h' = n + z*(h-n) = n + e. Downstream matmuls consume
  n and e separately (linear), so the h'=n+e add runs OFF the critical cycle.
- argmax feedback: reduce_max + is_ge directly on the fc PSUM, bf16 mask,
  PE transposes (bf16, 1 cyc/row), copy to SBUF one-hot.
- Raw logits: one strided DVE copy per step into a bf16 SBUF ring
  ([p, subtile, t%16, ch]); ring flushed to DRAM staging every 16 steps with
  1KB descriptors. Post-pass computes softmaxes per 128-batch subtile and
  writes fp16 outputs (host upcasts to fp32).
"""
import sys

sys.path.insert(0, "/opt/trn_rl_repo")
import numpy as np

import concourse.bass as bass
import concourse.bacc as bacc
import concourse.tile as tile
from concourse import mybir
from concourse import bass_utils
from concourse.tile import add_dep_helper
from concourse.mybir import AluOpType as Op
from concourse.mybir import ActivationFunctionType as AF

F32 = mybir.dt.float32
F32R = mybir.dt.float32r
BF16 = mybir.dt.bfloat16
FP16 = mybir.dt.float16
AX = mybir.AxisListType

H = 128
T = 256
NOUT = 33
NACT = 32
NCORES = 8
BCORE = 1024
G = 2                 # independent feedback groups per core
CH = BCORE // G       # batch per group
SUB = CH // H         # 128-batch subtiles per group
RING = 16             # steps per logits ring flush

# ablation flags (timing experiments; correctness requires dma/argmax/post True)
ABL = {"dma": True, "argmax": True, "post": True, "small_out": False}
# scheduling/balance tunables (all correctness-neutral)
TUNE = {"psg": 5, "psp": 2, "psm": 1, "work": 2, "ring_act": False,
        "oh_act": 0,  # oh_act: 0=DVE only, 1=split ACT/DVE, 2=ACT only
        "phase": 0,   # 0=none, 1=t0 offset, 2=per-step stagger
        "split1": True,   # wih1 consumes n0,e0 separately (h0-add off cycle)
        "eng_flip": False,  # group1 uses Pool for e, DVE for d/h-adds
        "chunks": 2,      # sub-chunks per group for latency pipelining
        "zfull": False}   # sigma-z at full group width (regression: keep False)

_cache = {}


def _bcast(ap, count):
    """Append a stride-0 innermost dim (free-dim broadcast read)."""
    return bass.AP(tensor=ap.tensor, offset=ap.offset, ap=list(ap.ap) + [[0, count]])


def build(steps=T):
    key = (steps, tuple(sorted(ABL.items())), tuple(sorted(TUNE.items())))
    if key in _cache:
        return _cache[key]
    nc = bacc.Bacc("TRN2", target_bir_lowering=False, debug=False,
                   num_devices=NCORES)

    din = {}
    for name, shape in [
        ("h0", [H, BCORE]), ("h1", [H, BCORE]),
        ("gi", [32, 384]),
        ("whh0", [H, 384]), ("wih1", [H, 384]), ("whh1", [H, 384]),
        ("fcw", [H, NOUT]), ("fcb4", [1, SUB * NOUT]), ("ones1", [1, H]),
        ("ident", [H, H]), ("biasc", [H, 8]),
    ]:
        din[name] = nc.dram_tensor(name, shape, F32, kind="ExternalInput")
    oshape = [1, 1, 1, 1] if ABL["small_out"] else [H, G * SUB, steps, NOUT]
    probs_o = nc.dram_tensor("probs16", oshape, FP16, kind="ExternalOutput")
    logp_o = nc.dram_tensor("logp16", oshape, FP16, kind="ExternalOutput")
    staging = nc.dram_tensor("staging", [H, G * SUB, steps, NOUT], BF16,
                             kind="Internal")
    NSUB = G * SUB  # total 128-batch subtiles per core (8)

    with tile.TileContext(nc) as tc:
        with tc.tile_pool(name="singles", bufs=1) as singles:
            h0 = [singles.tile([H, CH], F32R, tag=f"h0_{g}", name=f"h0_{g}") for g in range(G)]
            h1 = [singles.tile([H, CH], F32R, tag=f"h1_{g}", name=f"h1_{g}") for g in range(G)]
            gi = singles.tile([32, 384], F32R, tag="gi")
            whh0 = singles.tile([H, 384], F32R, tag="whh0")
            wih1 = singles.tile([H, 384], F32R, tag="wih1")
            whh1 = singles.tile([H, 384], F32R, tag="whh1")
            fcw = singles.tile([H, NOUT], F32, tag="fcw")
            fcb4 = singles.tile([1, SUB * NOUT], F32, tag="fcb4")
            ones1 = singles.tile([1, H], F32, tag="ones1")
            identr = singles.tile([H, H], F32R, tag="identr")
            identf = singles.tile([H, H], F32, tag="identf")
            identb = singles.tile([H, H], BF16, tag="identb")
            biasc = singles.tile([H, 8], F32, tag="biasc")
            for g in range(G):
                sl = slice(g * CH, (g + 1) * CH)
                nc.gpsimd.dma_start(h0[g][:], din["h0"][:, sl])
                nc.gpsimd.dma_start(h1[g][:], din["h1"][:, sl])
            for t_sb, name in [(gi, "gi"), (whh0, "whh0"), (wih1, "wih1"),
                               (whh1, "whh1"), (identr, "ident")]:
                nc.gpsimd.dma_start(t_sb[:], din[name][:])  # f32 -> f32r
            nc.sync.dma_start(fcw[:], din["fcw"][:])
            nc.sync.dma_start(fcb4[:], din["fcb4"][:])
            nc.sync.dma_start(ones1[:], din["ones1"][:])
            nc.sync.dma_start(identf[:], din["ident"][:])
            nc.sync.dma_start(biasc[:], din["biasc"][:])
            nc.vector.tensor_copy(identb[:], identf[:])

            with (
                tc.tile_pool(name="psg", bufs=TUNE["psg"], space="PSUM") as psg,
                tc.tile_pool(name="psp", bufs=TUNE["psp"], space="PSUM") as psp,
                tc.tile_pool(name="psm", bufs=TUNE["psm"], space="PSUM") as psm,
                tc.tile_pool(name="work", bufs=TUNE["work"]) as work,
                tc.tile_pool(name="oh_pool", bufs=2) as oh_pool,
                tc.tile_pool(name="ring_pool", bufs=2) as ring_pool,
            ):
                oh = [None] * G
                ring = None
                ringv = None
                tanh0_i = [None] * G
                for t in range(steps):
                    if ABL["dma"] and t % RING == 0:
                        ring = ring_pool.tile([H, NSUB * RING * NOUT], BF16,
                                              tag="ring")
                        ringv = ring[:].rearrange("p (i u c) -> p i u c",
                                                  u=RING, c=NOUT)
                    for g in range(G):
                        NC = TUNE["chunks"]
                        CW = CH // NC
                        flip = TUNE["eng_flip"] and g == 1
                        eng_d = nc.vector if flip else nc.gpsimd
                        eng_e = nc.gpsimd if flip else nc.vector
                        b_r0 = biasc[:, 0:1] if t == 0 else 0.0
                        b_z0 = biasc[:, 1:2] if t == 0 else 0.0
                        b_n0 = biasc[:, 2:3] if t == 0 else 0.0
                        r0 = psg.tile([H, CH], F32, tag="g")
                        z0 = psg.tile([H, CH], F32, tag="g")
                        pin0 = psg.tile([H, CH], F32, tag="g")
                        phn0 = psg.tile([H, CH], F32, tag="g")
                        r1 = psg.tile([H, CH], F32, tag="g")
                        z1 = psg.tile([H, CH], F32, tag="g")
                        pin1 = psg.tile([H, CH], F32, tag="g")
                        phn1 = psg.tile([H, CH], F32, tag="g")
                        rt0 = work.tile([H, CH], F32, tag="rt0")
                        zt0 = work.tile([H, CH], F32, tag="zt0")
                        tt0 = work.tile([H, CH], F32R, tag="tt0")
                        nt0 = work.tile([H, CH], F32R, tag="nt0")
                        dt0 = work.tile([H, CH], F32, tag="dt0")
                        et0 = work.tile([H, CH], F32R, tag="et0")
                        rt1 = work.tile([H, CH], F32, tag="rt1")
                        zt1 = work.tile([H, CH], F32, tag="zt1")
                        tt1 = work.tile([H, CH], F32R, tag="tt1")
                        nt1 = work.tile([H, CH], F32, tag="nt1")
                        dt1 = work.tile([H, CH], F32, tag="dt1")
                        et1 = work.tile([H, CH], F32, tag="et1")
                        for c in range(NC):
                            cs = slice(c * CW, (c + 1) * CW)
                            # ---------------- layer 0 chunk ----------------
                            nc.tensor.matmul(r0[:, cs], whh0[:, 0:128],
                                             h0[g][:, cs],
                                             start=True, stop=(t == 0))
                            nc.tensor.matmul(z0[:, cs], whh0[:, 128:256],
                                             h0[g][:, cs],
                                             start=True, stop=(t == 0))
                            nc.tensor.matmul(phn0[:, cs], whh0[:, 256:384],
                                             h0[g][:, cs],
                                             start=True, stop=True)
                            if t > 0 and ABL["argmax"]:
                                nc.tensor.matmul(r0[:, cs], gi[:, 0:128],
                                                 oh[g][:, cs],
                                                 start=False, stop=True)
                                nc.tensor.matmul(z0[:, cs], gi[:, 128:256],
                                                 oh[g][:, cs],
                                                 start=False, stop=True)
                                nc.tensor.matmul(pin0[:, cs], gi[:, 256:384],
                                                 oh[g][:, cs],
                                                 start=True, stop=False)
                            nc.scalar.activation(rt0[:, cs], r0[:, cs],
                                                 AF.Sigmoid, bias=b_r0)
                            if TUNE["zfull"]:
                                if c == NC - 1:
                                    nc.scalar.activation(zt0[:], z0[:],
                                                         AF.Sigmoid, bias=b_z0)
                            else:
                                nc.scalar.activation(zt0[:, cs], z0[:, cs],
                                                     AF.Sigmoid, bias=b_z0)
                            nc.vector.scalar_tensor_tensor(
                                out=tt0[:, cs], in0=phn0[:, cs],
                                scalar=biasc[:, 6:7],
                                in1=rt0[:, cs], op0=Op.add, op1=Op.mult)
                            nc.tensor.matmul(pin0[:, cs], identr[:], tt0[:, cs],
                                             start=(t == 0 or not ABL["argmax"]),
                                             stop=True)
                            nc.scalar.activation(nt0[:, cs], pin0[:, cs],
                                                 AF.Tanh, bias=b_n0)
                            eng_d.tensor_tensor(out=dt0[:, cs],
                                                in0=h0[g][:, cs].bitcast(F32),
                                                in1=nt0[:, cs].bitcast(F32),
                                                op=Op.subtract)
                            eng_e.tensor_tensor(out=et0[:, cs],
                                                in0=zt0[:, cs], in1=dt0[:, cs],
                                                op=Op.mult)
                            # h0 <- n0 + e0 (off critical path)
                            eng_d.tensor_tensor(out=h0[g][:, cs],
                                                in0=nt0[:, cs].bitcast(F32),
                                                in1=et0[:, cs].bitcast(F32),
                                                op=Op.add)
                            # ---------------- layer 1 chunk ----------------
                            nc.tensor.matmul(phn1[:, cs], whh1[:, 256:384],
                                             h1[g][:, cs],
                                             start=True, stop=True)
                            nc.tensor.matmul(r1[:, cs], whh1[:, 0:128],
                                             h1[g][:, cs],
                                             start=True, stop=False)
                            nc.tensor.matmul(z1[:, cs], whh1[:, 128:256],
                                             h1[g][:, cs],
                                             start=True, stop=False)
                            if TUNE["split1"]:
                                nc.tensor.matmul(r1[:, cs], wih1[:, 0:128],
                                                 nt0[:, cs],
                                                 start=False, stop=False)
                                nc.tensor.matmul(z1[:, cs], wih1[:, 128:256],
                                                 nt0[:, cs],
                                                 start=False, stop=False)
                                nc.tensor.matmul(pin1[:, cs], wih1[:, 256:384],
                                                 nt0[:, cs],
                                                 start=True, stop=False)
                                nc.tensor.matmul(r1[:, cs], wih1[:, 0:128],
                                                 et0[:, cs],
                                                 start=False, stop=True)
                                nc.tensor.matmul(z1[:, cs], wih1[:, 128:256],
                                                 et0[:, cs],
                                                 start=False, stop=True)
                                nc.tensor.matmul(pin1[:, cs], wih1[:, 256:384],
                                                 et0[:, cs],
                                                 start=False, stop=False)
                            else:
                                nc.tensor.matmul(r1[:, cs], wih1[:, 0:128],
                                                 h0[g][:, cs],
                                                 start=False, stop=True)
                                nc.tensor.matmul(z1[:, cs], wih1[:, 128:256],
                                                 h0[g][:, cs],
                                                 start=False, stop=True)
                                nc.tensor.matmul(pin1[:, cs], wih1[:, 256:384],
                                                 h0[g][:, cs],
                                                 start=True, stop=False)
                            nc.scalar.activation(rt1[:, cs], r1[:, cs],
                                                 AF.Sigmoid, bias=biasc[:, 3:4])
                            if TUNE["zfull"]:
                                if c == NC - 1:
                                    nc.scalar.activation(zt1[:], z1[:],
                                                         AF.Sigmoid,
                                                         bias=biasc[:, 4:5])
                            else:
                                nc.scalar.activation(zt1[:, cs], z1[:, cs],
                                                     AF.Sigmoid,
                                                     bias=biasc[:, 4:5])
                            nc.vector.scalar_tensor_tensor(
                                out=tt1[:, cs], in0=phn1[:, cs],
                                scalar=biasc[:, 7:8],
                                in1=rt1[:, cs], op0=Op.add, op1=Op.mult)
                            nc.tensor.matmul(pin1[:, cs], identr[:], tt1[:, cs],
                                             start=False, stop=True)
                            nc.scalar.activation(nt1[:, cs], pin1[:, cs],
                                                 AF.Tanh, bias=biasc[:, 5:6])
                            eng_d.tensor_tensor(out=dt1[:, cs],
                                                in0=h1[g][:, cs].bitcast(F32),
                                                in1=nt1[:, cs],
                                                op=Op.subtract)
                            eng_e.tensor_tensor(out=et1[:, cs],
                                                in0=zt1[:, cs], in1=dt1[:, cs],
                                                op=Op.mult)
                            eng_d.tensor_tensor(out=h1[g][:, cs],
                                                in0=nt1[:, cs], in1=et1[:, cs],
                                                op=Op.add)
                        # ---------------- fc + argmax ----------------
                        pred = psp.tile([H, SUB * NOUT], F32, tag="pred")
                        nc.tensor.matmul(pred[:], ones1[:], fcb4[:],
                                         start=True, stop=False)
                        for i in range(SUB):
                            osl = slice(i * NOUT, (i + 1) * NOUT)
                            bsl = slice(i * H, (i + 1) * H)
                            nc.tensor.matmul(pred[:, osl], nt1[:, bsl], fcw[:],
                                             start=False, stop=False)
                            nc.tensor.matmul(pred[:, osl], et1[:, bsl], fcw[:],
                                             start=False, stop=(i == SUB - 1))
                        pred3 = pred[:].rearrange("p (i c) -> p i c", c=NOUT)
                        if ABL["dma"]:
                            if TUNE["ring_act"]:
                                nc.scalar.copy(
                                    ringv[:, SUB * g:SUB * (g + 1), t % RING, :],
                                    pred3)
                            else:
                                nc.vector.tensor_copy(
                                    ringv[:, SUB * g:SUB * (g + 1), t % RING, :],
                                    pred3)
                        if ABL["argmax"] and t + 1 < steps:
                            mx = work.tile([H, SUB], F32, tag="mx")
                            nc.vector.reduce_max(mx[:], pred3[:, :, 0:NACT],
                                                 axis=AX.X)
                            mask = work.tile([H, SUB * NACT], BF16, tag="mask")
                            mask3 = mask[:].rearrange("p (i c) -> p i c", c=NACT)
                            nc.vector.tensor_tensor(out=mask3,
                                                    in0=pred3[:, :, 0:NACT],
                                                    in1=_bcast(mx[:], NACT),
                                                    op=Op.is_ge)
                            ohp = psm.tile([32, CH], BF16, tag="ohT")
                            for j in range(SUB):
                                nc.tensor.matmul(
                                    ohp[:, j * H:(j + 1) * H],
                                    mask[:, j * NACT:(j + 1) * NACT],
                                    identb[:], is_transpose=True,
                                    start=(j == 0), stop=(j == SUB - 1))
                            oh[g] = oh_pool.tile([32, CH], F32R, tag=f"oh{g}", name=f"oh{g}")
                            if TUNE["oh_act"] == 0:
                                nc.vector.tensor_copy(oh[g][:], ohp[:])
                            elif TUNE["oh_act"] == 2:
                                nc.scalar.copy(oh[g][:], ohp[:])
                            else:
                                nc.scalar.copy(oh[g][:, 0:CH // 2],
                                               ohp[:, 0:CH // 2])
                                nc.vector.tensor_copy(oh[g][:, CH // 2:CH],
                                                      ohp[:, CH // 2:CH])
                    if ABL["dma"] and (t % RING == RING - 1 or t == steps - 1):
                        t0 = t - t % RING
                        nc.sync.dma_start(
                            staging[:, :, t0:t + 1, :],
                            ringv[:, :, 0:t % RING + 1, :])

            # ---------------- post-pass: softmaxes ----------------
            with (
                tc.tile_pool(name="post_in", bufs=2) as post_in,
                tc.tile_pool(name="post_et", bufs=2) as post_et,
                tc.tile_pool(name="post_o", bufs=2) as post_o,
                tc.tile_pool(name="small", bufs=2) as small,
            ):
                for i in range(NSUB if (ABL["post"] and not ABL["small_out"]) else 0):
                    pt = post_in.tile([H, steps * NOUT], BF16, tag="pt")
                    pt3 = pt[:].rearrange("p (t c) -> p t c", c=NOUT)
                    nc.sync.dma_start(pt3, staging[:, i])
                    et = post_et.tile([H, steps * NOUT], BF16, tag="et")
                    et3 = et[:].rearrange("p (t c) -> p t c", c=NOUT)
                    nc.scalar.activation(et[:], pt[:], AF.Exp)
                    s = small.tile([H, steps], F32, tag="s")
                    nc.vector.reduce_sum(s[:], et3[:, :, 0:NACT], axis=AX.X)
                    rs = small.tile([H, steps], F32, tag="rs")
                    nc.vector.reciprocal(rs[:], s[:])
                    ls = small.tile([H, steps], F32, tag="ls")
                    nc.scalar.activation(ls[:], s[:], AF.Ln)
                    # duration channel (softmax over time)
                    dr = small.tile([H, steps], F32, tag="dr")
                    nc.vector.tensor_copy(dr[:], pt3[:, :, NACT])
                    de = small.tile([H, steps], F32, tag="de")
                    dsum = small.tile([H, 1], F32, tag="dsum")
                    nc.scalar.activation(de[:], dr[:], AF.Exp, accum_out=dsum[:])
                    drs = small.tile([H, 1], F32, tag="drs")
                    nc.vector.reciprocal(drs[:], dsum[:])
                    dv = small.tile([H, steps], FP16, tag="dv")
                    nc.vector.tensor_scalar_mul(dv[:], de[:], drs[:])
                    pb = post_o.tile([H, steps * NOUT], FP16, tag="pb")
                    pb3 = pb[:].rearrange("p (t c) -> p t c", c=NOUT)
                    lb = post_o.tile([H, steps * NOUT], FP16, tag="lb")
                    lb3 = lb[:].rearrange("p (t c) -> p t c", c=NOUT)
                    nc.vector.tensor_tensor(out=pb3, in0=et3,
                                            in1=_bcast(rs[:], NOUT), op=Op.mult)
                    nc.gpsimd.tensor_tensor(out=lb3, in0=pt3,
                                             in1=_bcast(ls[:], NOUT),
                                             op=Op.subtract)
                    nc.vector.tensor_copy(pb3[:, :, NACT], dv[:])
                    nc.gpsimd.tensor_copy(lb3[:, :, NACT], dv[:])
                    nc.sync.dma_start(probs_o[:, i], pb3)
                    nc.sync.dma_start(logp_o[:, i], lb3)

    nc.compile()
    _cache[key] = nc
    return nc


def host_precompute(emb, w_ih_0, w_hh_0, b_ih_0, b_hh_0, w_ih_1, w_hh_1,
                    b_ih_1, b_hh_1, fc_w, fc_b):
    f = np.float32
    emb = np.asarray(emb, f)
    w_ih_0, w_hh_0 = np.asarray(w_ih_0, f), np.asarray(w_hh_0, f)
    b_ih_0, b_hh_0 = np.asarray(b_ih_0, f), np.asarray(b_hh_0, f)
    w_ih_1, w_hh_1 = np.asarray(w_ih_1, f), np.asarray(w_hh_1, f)
    b_ih_1, b_hh_1 = np.asarray(b_ih_1, f), np.asarray(b_hh_1, f)
    fc_w, fc_b = np.asarray(fc_w, f), np.asarray(fc_b, f)

    emb_ext = np.concatenate([emb, np.ones((NACT, 1), f)], 1)
    GI = (emb_ext @ w_ih_0.T + b_ih_0).astype(f)
    GI[:, 0:128] += b_hh_0[0:128]
    GI[:, 128:256] += b_hh_0[128:256]
    x0 = np.concatenate([emb[0], np.zeros(1, f)])
    gi0 = (x0 @ w_ih_0.T + b_ih_0).astype(f)
    gi0[0:128] += b_hh_0[0:128]
    gi0[128:256] += b_hh_0[128:256]

    def wT(w):
        return np.concatenate([w[0:128].T, w[128:256].T, w[256:384].T], 1).astype(f)

    biasc = np.stack([
        gi0[0:128], gi0[128:256], gi0[256:384],
        (b_ih_1[0:128] + b_hh_1[0:128]).astype(f),
        (b_ih_1[128:256] + b_hh_1[128:256]).astype(f),
        b_ih_1[256:384], b_hh_0[256:384], b_hh_1[256:384],
    ], axis=1).astype(f)

    return {
        "gi": GI.astype(f),
        "whh0": wT(w_hh_0), "wih1": wT(w_ih_1), "whh1": wT(w_hh_1),
        "fcw": fc_w.T.copy(), "fcb4": np.tile(fc_b, SUB)[None, :].astype(f),
        "ones1": np.ones((1, H), f), "ident": np.eye(H, dtype=f),
        "biasc": biasc,
    }


def kernel(batch_size, hidden, emb, w_ih_0, w_hh_0, b_ih_0, b_hh_0,
           w_ih_1, w_hh_1, b_ih_1, b_hh_1, fc_w, fc_b):
    hidden = np.asarray(hidden, np.float32)
    B = hidden.shape[1]
    assert B == NCORES * BCORE, f"unexpected batch {B}"
    consts = host_precompute(emb, w_ih_0, w_hh_0, b_ih_0, b_hh_0,
                             w_ih_1, w_hh_1, b_ih_1, b_hh_1, fc_w, fc_b)
    nc = build(T)
    in_maps = []
    for i in range(NCORES):
        sl = slice(i * BCORE, (i + 1) * BCORE)
        m = dict(consts)
        m["h0"] = np.ascontiguousarray(hidden[0, sl].T)
        m["h1"] = np.ascontiguousarray(hidden[1, sl].T)
        in_maps.append(m)
    res = bass_utils.run_bass_kernel_spmd(nc, in_maps, core_ids=list(range(NCORES)))

    def unpack(name):
        parts = []
        for i in range(NCORES):
            a = np.asarray(res.results[i][name])  # [H, 8, T, NOUT] fp16
            parts.append(np.transpose(a, (1, 0, 2, 3)).reshape(BCORE, T, NOUT))
        return np.concatenate(parts, 0).astype(np.float32)

    return unpack("logp16"), unpack("probs16")
